# revision 5
# baseline (speedup 1.0000x reference)
"""Epipolar (KNN-sparse) attention on 8 Trainium2 NeuronCores.

Problem (full shapes): B=2, HW=4096, NTGT=4096, C=512, H=8 heads, DH=64, KNN=32.
  q = src@Wq+bq ; k = tgt@Wk+bk ; v = tgt@Wv+bv     (per-head split of C)
  k_g/v_g = gather of KNN target rows per query; logits = q.k_g * DH^-.5 + w
  out = softmax(logits) @ v_g ; return out @ Wo + bo

Sharding: 8 cores = 2 batches x 4 head-pairs (2 heads = 128 channels each).
Per core: project q/k/v for its 128-channel slice, gather k||v rows (one
512B-per-row dma_gather per 128-query tile), softmax-attend on DVE/ACT/Pool,
out-project against Wo[slice] -> partial (4096,512).  Host sums the 4 row-
parallel partials per batch (standard tensor-parallel unshard) and adds
nothing else (biases are folded on-device; bo is added as bo/4 on each core).

Numerics: projections/gather/attention in bf16 with f32 accumulation
(PE psum, f32 tensor_reduce tail, f32 exp/denominator).  Measured emulation
error vs f32 reference: ~7e-3 rms.
"""

import os
import sys

import numpy as np

sys.path.insert(0, "/opt/trn_rl_repo")

from contextlib import ExitStack

import concourse.bass as bass
import concourse.tile as tile
from concourse import bacc, masks, mybir
from concourse.bass_utils import run_bass_kernel_spmd

F32 = mybir.dt.float32
BF16 = mybir.dt.bfloat16
I16 = mybir.dt.int16
AF = mybir.ActivationFunctionType
OP = mybir.AluOpType

# full-problem constants
B, HW, NTGT, C = 2, 4096, 4096, 512
H, KNN = 8, 32
DH = C // H
SCALE = DH ** -0.5
HL = 2                 # heads per core
DHL = HL * DH          # 128 local channels
P = 128                # partitions / queries per tile


def build_program(hw=HW, ntgt=NTGT):
    """Emit the SPMD single-core program (identical on all 8 cores)."""
    nt = hw // P            # query tiles
    ntt = ntgt // P         # target tiles
    ck = C // P             # contraction chunks (4)

    nc = bacc.Bacc("TRN2", target_bir_lowering=False, debug=False,
                   num_devices=8)

    srcT = nc.dram_tensor("srcT", (C, hw), F32, kind="ExternalInput").ap()
    tgtT = nc.dram_tensor("tgtT", (C, ntgt), F32, kind="ExternalInput").ap()
    wq = nc.dram_tensor("wq", (C, DHL), F32, kind="ExternalInput").ap()
    wk = nc.dram_tensor("wk", (C, DHL), F32, kind="ExternalInput").ap()
    wv = nc.dram_tensor("wv", (C, DHL), F32, kind="ExternalInput").ap()
    wo = nc.dram_tensor("wo", (DHL, C), F32, kind="ExternalInput").ap()
    bq = nc.dram_tensor("bq", (1, DHL), F32, kind="ExternalInput").ap()
    bk = nc.dram_tensor("bk", (1, DHL), F32, kind="ExternalInput").ap()
    bv = nc.dram_tensor("bv", (1, DHL), F32, kind="ExternalInput").ap()
    bo4 = nc.dram_tensor("bo4", (1, C), F32, kind="ExternalInput").ap()
    idxw = nc.dram_tensor("idxw", (nt, P, KNN * P // 16), I16,
                          kind="ExternalInput").ap()
    wts = nc.dram_tensor("wts", (hw, KNN), F32, kind="ExternalInput").ap()
    out = nc.dram_tensor("out", (hw, C), F32, kind="ExternalOutput").ap()

    with tile.TileContext(nc) as tc, ExitStack() as ctx:
        tp = lambda name, bufs, **kw: ctx.enter_context(
            tc.tile_pool(name=name, bufs=bufs, **kw))

        cpool = tp("consts", 1)
        dram = tp("dram", 1, space="DRAM")
        kv_dram = dram.tile([ntgt, 2 * DHL], BF16)

        ident = cpool.tile([P, P], BF16, tag="ident")
        masks.make_identity(nc, ident[:])
        ones = cpool.tile([1, P], BF16, tag="ones")
        nc.gpsimd.memset(ones[:], 1.0)

        # weights + biases resident in bf16
        wq_sb = cpool.tile([P, ck * DHL], BF16, tag="wq")
        wk_sb = cpool.tile([P, ck * DHL], BF16, tag="wk")
        wv_sb = cpool.tile([P, ck * DHL], BF16, tag="wv")
        wo_sb = cpool.tile([P, C], BF16, tag="wo")
        bq_sb = cpool.tile([1, DHL], BF16, tag="bq")
        bk_sb = cpool.tile([1, DHL], BF16, tag="bk")
        bv_sb = cpool.tile([1, DHL], BF16, tag="bv")
        bo_sb = cpool.tile([1, C], BF16, tag="bo")
        srcT_sb = cpool.tile([P, ck * hw], BF16, tag="srcT")

        with tc.tile_pool(name="wstage", bufs=2) as wstg:
            for w_hbm, w_sb in ((wq, wq_sb), (wk, wk_sb), (wv, wv_sb)):
                for c in range(ck):
                    st = wstg.tile([P, DHL], F32, tag="wchunk")
                    nc.sync.dma_start(st[:], w_hbm[c * P:(c + 1) * P, :])
                    nc.vector.tensor_copy(
                        w_sb[:, c * DHL:(c + 1) * DHL], st[:])
            st = wstg.tile([P, C], F32, tag="wochunk")
            nc.sync.dma_start(st[:], wo[:, :])
            nc.vector.tensor_copy(wo_sb[:], st[:])
            for b_hbm, b_sb in ((bq, bq_sb), (bk, bk_sb), (bv, bv_sb),
                                (bo4, bo_sb)):
                stb = wstg.tile([1, C], F32, tag="bchunk")
                n = b_hbm.shape[1]
                nc.sync.dma_start(stb[:1, :n], b_hbm[:, :])
                nc.vector.tensor_copy(b_sb[:1, :n], stb[:1, :n])

        # ---- phase 1: tgt load + k/v projection -> kv_dram ----
        with tc.tile_pool(name="tgtT", bufs=1) as tpool, \
             tc.tile_pool(name="stage", bufs=2) as stg, \
             tc.tile_pool(name="p1psum", bufs=4, space="PSUM") as p1ps, \
             tc.tile_pool(name="p1out", bufs=3) as p1out:
            tgtT_sb = tpool.tile([P, ck * ntgt], BF16, tag="tgtT")
            for c in range(ck):
                st = stg.tile([P, ntgt], F32, tag="ldchunk")
                nc.sync.dma_start(st[:], tgtT[c * P:(c + 1) * P, :])
                nc.vector.tensor_copy(
                    tgtT_sb[:, c * ntgt:(c + 1) * ntgt], st[:])
            for c in range(ck):
                st = stg.tile([P, hw], F32, tag="ldchunk")
                nc.sync.dma_start(st[:], srcT[c * P:(c + 1) * P, :])
                nc.scalar.copy(srcT_sb[:, c * hw:(c + 1) * hw], st[:])

            for t in range(ntt):
                psk = p1ps.tile([P, DHL], F32, tag="psk")
                psv = p1ps.tile([P, DHL], F32, tag="psv")
                for c in range(ck):
                    lhsT = tgtT_sb[:, c * ntgt + t * P: c * ntgt + (t + 1) * P]
                    nc.tensor.matmul(psk[:], lhsT,
                                     wk_sb[:, c * DHL:(c + 1) * DHL],
                                     start=(c == 0), stop=False)
                    nc.tensor.matmul(psv[:], lhsT,
                                     wv_sb[:, c * DHL:(c + 1) * DHL],
                                     start=(c == 0), stop=False)
                nc.tensor.matmul(psk[:], ones[:1, :], bk_sb[:1, :],
                                 start=False, stop=True)
                nc.tensor.matmul(psv[:], ones[:1, :], bv_sb[:1, :],
                                 start=False, stop=True)
                kv_sb = p1out.tile([P, 2 * DHL], BF16, tag="kv")
                nc.scalar.copy(kv_sb[:, 0:DHL], psk[:])
                nc.scalar.copy(kv_sb[:, DHL:2 * DHL], psv[:])
                nc.sync.dma_start(kv_dram[t * P:(t + 1) * P, :], kv_sb[:])

        # ---- phase 2: per-query-tile sparse attention ----
        qps = tp("qpsum", 2, space="PSUM")
        tps = tp("tpsum", 2, space="PSUM")
        ops_pool = tp("opsum", 2, space="PSUM")
        small = tp("small", 3)
        gat = tp("gather", 2)
        big = tp("big", 2)
        outp = tp("outstage", 2)

        for t in range(nt):
            # q projection (PE)
            psq = qps.tile([P, DHL], F32, tag="psq")
            for c in range(ck):
                nc.tensor.matmul(
                    psq[:],
                    srcT_sb[:, c * hw + t * P: c * hw + (t + 1) * P],
                    wq_sb[:, c * DHL:(c + 1) * DHL],
                    start=(c == 0), stop=False)
            nc.tensor.matmul(psq[:], ones[:1, :], bq_sb[:1, :],
                             start=False, stop=True)
            q_sb = small.tile([P, DHL], BF16, tag="q")
            nc.scalar.copy(q_sb[:], psq[:])

            # gather k||v rows for this tile (512B per row, j-major order)
            idx_sb = small.tile([P, KNN * P // 16], I16, tag="idx")
            nc.sync.dma_start(idx_sb[:], idxw[t, :, :])
            w_sb = small.tile([P, KNN], F32, tag="w")
            nc.sync.dma_start(w_sb[:], wts[t * P:(t + 1) * P, :])
            kvg = gat.tile([P, KNN * 2 * DHL], BF16, tag="kvg")
            nc.gpsimd.dma_gather(
                kvg[:].rearrange("p (j d) -> p j d", j=KNN),
                kv_dram[:, :],
                idx_sb[:],
                num_idxs=KNN * P,
                num_idxs_reg=KNN * P,
                elem_size=2 * DHL,
                single_packet=False,
            )

            kvg3 = kvg[:].rearrange("p (j d) -> p j d", j=KNN)
            kg = kvg3[:, :, 0:DHL]                       # [P, KNN, 128]
            vg = kvg3[:, :, DHL:2 * DHL]

            # qk product (DVE, bf16 2x): prod[p, j, d] = kg * q
            prod = big.tile([P, KNN * DHL], BF16, tag="prod")
            prod3 = prod[:].rearrange("p (j d) -> p j d", j=KNN)
            qb = q_sb[:].unsqueeze(1).broadcast_to([P, KNN, DHL])
            nc.vector.tensor_tensor(prod3, kg, qb, op=OP.mult)

            # 3-level bf16 pair-tree over d: 128 -> 16 per j
            tr_in = prod
            width = DHL
            for lvl in range(3):
                width //= 2
                tr = big.tile([P, KNN * width], BF16, tag=f"tr{lvl}")
                a = tr_in[:].rearrange("p (j d two) -> p j d two", j=KNN, two=2)
                nc.vector.tensor_tensor(
                    tr[:].rearrange("p (j d) -> p j d", j=KNN),
                    a[:, :, :, 0], a[:, :, :, 1], op=OP.add)
                tr_in = tr

            # f32 reduce of the remaining 16 per (j,h) -> logits [P, KNN, HL]
            logits = small.tile([P, KNN * HL], F32, tag="logits")
            nc.vector.tensor_reduce(
                logits[:].rearrange("p (j h) -> p j h", j=KNN),
                tr_in[:].rearrange("p (j h e) -> p j h e", j=KNN, h=HL),
                axis=mybir.AxisListType.X, op=OP.add)

            # + pair weights (broadcast across heads)
            logw = small.tile([P, KNN * HL], F32, tag="logw")
            nc.vector.tensor_tensor(
                logw[:].rearrange("p (j h) -> p j h", j=KNN),
                logits[:].rearrange("p (j h) -> p j h", j=KNN),
                w_sb[:].unsqueeze(2).broadcast_to([P, KNN, HL]),
                op=OP.add)

            # exp + per-head denominator (ACT, fused accumulate)
            ex = small.tile([P, HL * KNN], BF16, tag="ex")   # h-major
            den = small.tile([P, HL], F32, tag="den")
            logw3 = logw[:].rearrange("p (j h) -> p h j", h=HL)
            for h in range(HL):
                nc.scalar.activation(
                    ex[:, h * KNN:(h + 1) * KNN], logw3[:, h, :], AF.Exp,
                    accum_out=den[:, h:h + 1])
            rec = small.tile([P, HL], F32, tag="rec")
            nc.vector.reciprocal(rec[:], den[:])

            # v weighting (Pool, bf16): vprod[p, j, h, d] = vg * ex
            vprod = big.tile([P, KNN * DHL], BF16, tag="vprod")
            exb = (ex[:].rearrange("p (h j) -> p j h", h=HL)
                   .unsqueeze(3).broadcast_to([P, KNN, HL, DH]))
            nc.gpsimd.tensor_tensor(
                vprod[:].rearrange("p (j h d) -> p j h d", j=KNN, h=HL),
                vg.rearrange("p j (h d) -> p j h d", h=HL),
                exb, op=OP.mult)

            # 5-level bf16 pair-tree over j: KNN -> 1
            vt_in = vprod
            jw = KNN
            for lvl in range(5):
                jw //= 2
                vt = big.tile([P, jw * DHL], BF16, tag=f"vt{lvl}")
                a = vt_in[:].rearrange("p (j two d) -> p j two d",
                                       two=2, d=DHL)
                nc.vector.tensor_tensor(
                    vt[:].rearrange("p (j d) -> p j d", d=DHL),
                    a[:, :, 0, :], a[:, :, 1, :], op=OP.add)
                vt_in = vt

            # normalize by denominator (ACT copy with per-partition scale)
            ao = small.tile([P, DHL], BF16, tag="ao")
            for h in range(HL):
                nc.scalar.activation(
                    ao[:, h * DH:(h + 1) * DH], vt_in[:, h * DH:(h + 1) * DH],
                    AF.Copy, scale=rec[:, h:h + 1])

            # transpose + output projection (PE)
            aoT_ps = tps.tile([P, P], BF16, tag="aoT")
            nc.tensor.transpose(aoT_ps[:], ao[:], ident[:])
            aoT = small.tile([P, P], BF16, tag="aoTsb")
            nc.scalar.copy(aoT[:], aoT_ps[:])
            ops = ops_pool.tile([P, C], F32, tag="ops")
            nc.tensor.matmul(ops[:], aoT[:], wo_sb[:], start=True, stop=False)
            nc.tensor.matmul(ops[:], ones[:1, :], bo_sb[:1, :],
                             start=False, stop=True)
            o_sb = outp.tile([P, C], F32, tag="osb")
            nc.scalar.copy(o_sb[:], ops[:])
            nc.sync.dma_start(out[t * P:(t + 1) * P, :], o_sb[:])

    nc.compile()
    return nc


def _wrap_indices(idx_b, hw=HW):
    """(hw, KNN) int -> (nt, 128, 256) int16, j-major per tile, 16-wrapped,
    replicated across the 8 gpsimd cores."""
    nt = hw // P
    out = np.empty((nt, P, KNN * P // 16), np.int16)
    for t in range(nt):
        flat = idx_b[t * P:(t + 1) * P, :].T.reshape(-1)   # L[j*128+q]
        wr = flat.reshape(-1, 16).T.astype(np.int16)       # [16, 256]
        out[t] = np.tile(wr, (8, 1))
    return out


_NC_CACHE = {}


def _get_program():
    if "nc" not in _NC_CACHE:
        _NC_CACHE["nc"] = build_program()
    return _NC_CACHE["nc"]


def make_in_maps(src, tgt, indices, weights, Wq, bq, Wk, bk, Wv, bv, Wo, bo,
                 hw=HW):
    f32 = np.float32
    src = np.asarray(src, f32)
    tgt = np.asarray(tgt, f32)
    weights = np.asarray(weights, f32)
    wqs = np.asarray(Wq, f32) * np.float32(SCALE)
    bqs = np.asarray(bq, f32) * np.float32(SCALE)
    in_maps = []
    for core in range(8):
        b, g = divmod(core, 4)
        hs = g * DHL
        m = {
            "srcT": np.ascontiguousarray(src[b].T),
            "tgtT": np.ascontiguousarray(tgt[b].T),
            "wq": np.ascontiguousarray(wqs[:, hs:hs + DHL]),
            "wk": np.ascontiguousarray(np.asarray(Wk, f32)[:, hs:hs + DHL]),
            "wv": np.ascontiguousarray(np.asarray(Wv, f32)[:, hs:hs + DHL]),
            "wo": np.ascontiguousarray(np.asarray(Wo, f32)[hs:hs + DHL, :]),
            "bq": np.ascontiguousarray(bqs[hs:hs + DHL]).reshape(1, DHL),
            "bk": np.ascontiguousarray(
                np.asarray(bk, f32)[hs:hs + DHL]).reshape(1, DHL),
            "bv": np.ascontiguousarray(
                np.asarray(bv, f32)[hs:hs + DHL]).reshape(1, DHL),
            "bo4": (np.asarray(bo, f32) / 4.0).reshape(1, C),
            "idxw": _wrap_indices(np.asarray(indices)[b], hw=hw),
            "wts": np.ascontiguousarray(weights[b]),
        }
        in_maps.append(m)
    return in_maps


def kernel(src, tgt, indices, weights, Wq, bq, Wk, bk, Wv, bv, Wo, bo):
    nc = _get_program()
    in_maps = make_in_maps(src, tgt, indices, weights,
                           Wq, bq, Wk, bk, Wv, bv, Wo, bo)
    res = run_bass_kernel_spmd(nc, in_maps, core_ids=list(range(8)))
    out = np.zeros((B, HW, C), np.float32)
    for core in range(8):
        out[core // 4] += res.results[core]["out"]
    return out
